# revision 22
# baseline (speedup 1.0000x reference)
"""Trainium2 Bass kernel for nn_BaseModel_2654289789315 (gnn_message_passing).

Restructured v1 (from 160us baseline):
  - Pair phase: geometry in fp16 where 2x DVE modes apply, polynomial cutoff
    (no Sin table), d/invd via one Ln + one Exp, so the whole pair phase uses
    only the {ln,exp} activation-table set. Atom phase uses only {silu,
    identity} -> 2 activation table loads total (baseline had 18).
  - One-hot scatter matrix split between GpSimd (LocalScatter) and DVE
    (is_equal vs iota) so neither engine is the sole bottleneck.
  - s-major one-hot columns (species*32 + atom_rel).
  - Scatter matmuls accumulate 4 blocks per PSUM bank -> 10 batched copies.
  - x0e stage via host-precomputed e3 table (no matmuls, one big DVE mult).
  - Atom phase: PSUM->SBUF copies split scalar/DVE, single big fp16 product,
    tree adds, stt fusions.

Sharding: atoms (and their incident pairs, grouped by center block) sharded
across 8 cores; weights replicated; per-pair endpoint positions materialized
host-side (the "halo exchange" happens at input marshaling).
"""

import sys
if "/opt/trn_rl_repo" not in sys.path:
    sys.path.insert(0, "/opt/trn_rl_repo")

import math
import numpy as np

import concourse.bass as bass
import concourse.mybir as mybir
import concourse.tile as tile
from concourse import bacc, bass_utils

AF = mybir.ActivationFunctionType
ALU = mybir.AluOpType
DT = mybir.dt

# ---- problem constants ----
N_ATOMS = 10000
N_PAIRS = 160000
N_TYPES = 4
N_CHANNELS = 32
N_MAX = 4
N_BASIS = 8
K = 128
CUTOFF = 20.0
CUTOFF_WIDTH = 5.0
MP_SCALING = 0.1
K0_TOT = 384
NCORES = 8
NLOC = N_ATOMS // NCORES          # 1250
A_BLK = 32
NBLK = NLOC // A_BLK + (1 if NLOC % A_BLK else 0)  # 40
NS = NBLK * A_BLK                  # 1280
P = 128
SQ3 = float(np.sqrt(3.0))
SIGMA = CUTOFF / N_BASIS
L_OF_LM = [0, 1, 1, 1, 2, 2, 2, 2, 2]
PI = float(np.pi)
FC_A = PI * PI / 4.0
FC_B = PI ** 4 / 48.0
FC_C = PI ** 6 / 1440.0

NCH = 2                            # pair chunks
CHB = NBLK // NCH                  # 20 blocks per chunk
# one-hot split: first KD tiles of each chunk on DVE, rest on gpsimd
KD_FRAC = 0.17
WIN = 14                           # LocalScatter window (tiles)

_BUILD_CACHE = {}


def _onehot_plan(TPB):
    """Per chunk: KD tiles for DVE is_equal; remaining in gpsimd windows."""
    TC = CHB * TPB
    KD = int(round(TC * KD_FRAC))
    ngp = TC - KD
    nwin = math.ceil(ngp / WIN)
    wts = [min(WIN, ngp - i * WIN) for i in range(nwin)]
    return TC, KD, wts


def _build(TPB):
    T = NBLK * TPB
    TC, KD, wts = _onehot_plan(TPB)
    NW = len(wts) * NCH

    nc = bacc.Bacc("TRN2", target_bir_lowering=False, debug=False,
                   num_devices=NCORES)

    def din(name, shape, dt=DT.float32):
        return nc.dram_tensor(name, shape, dt, kind="ExternalInput")

    rvin_d = din("rvin", [P, 3 * T])
    colf_d = din("colf", [P, T], DT.float16)
    idx16_d = din("idx16", [P, max(NW, 1) * WIN], DT.int16)
    iota16_d = din("iota16", [P, P], DT.float16)
    mu8_d = din("mu8", [P, N_BASIS])
    mp_d = din("mp", [32, 9 * K], DT.float16)
    mw_d = din("mw", [32, 9 * K], DT.float16)
    mpP_d = din("mpP", [32, 9 * K], DT.float16)
    mpM_d = din("mpM", [32, 9 * K], DT.float16)
    e3_d = din("e3", [K, 3 * NS], DT.float16)
    whead_d = din("whead", [K, 3 * K0_TOT], DT.float16)
    bhead_d = din("bhead", [K, 3])
    wout_d = din("wout", [K, 3], DT.float16)
    bout_d = din("bout", [1, 1])
    out_d = nc.dram_tensor("out", [1, NS], DT.float32, kind="ExternalOutput")

    f32 = DT.float32
    f16 = DT.float16

    with tile.TileContext(nc) as tc:
        with tc.tile_pool(name="const", bufs=1) as cp, \
             tc.tile_pool(name="gpool", bufs=1) as gp, \
             tc.tile_pool(name="pair", bufs=2) as wp, \
             tc.tile_pool(name="stp", bufs=2) as sp, \
             tc.tile_pool(name="atom", bufs=2) as ap, \
             tc.tile_pool(name="atom2", bufs=1) as ap2, \
             tc.tile_pool(name="psum", bufs=2, space="PSUM") as pp:

            # ---- inputs first (critical path), then weights ----
            rvin = gp.tile([P, 3 * T], f32)
            nc.sync.dma_start(rvin[:], rvin_d.ap())
            colf = cp.tile([P, T], f16)
            nc.scalar.dma_start(colf[:], colf_d.ap())
            idx16 = cp.tile([P, max(NW, 1) * WIN], DT.int16)
            nc.scalar.dma_start(idx16[:], idx16_d.ap())
            iota16 = cp.tile([P, P], f16)
            nc.scalar.dma_start(iota16[:], iota16_d.ap())
            mu8 = cp.tile([P, N_BASIS], f32)
            nc.sync.dma_start(mu8[:], mu8_d.ap())
            mp_sb = cp.tile([32, 9 * K], f16)
            nc.sync.dma_start(mp_sb[:], mp_d.ap())
            mw_sb = cp.tile([32, 9 * K], f16)
            nc.sync.dma_start(mw_sb[:], mw_d.ap())
            mpP_sb = cp.tile([32, 9 * K], f16)
            nc.sync.dma_start(mpP_sb[:], mpP_d.ap())
            mpM_sb = cp.tile([32, 9 * K], f16)
            nc.sync.dma_start(mpM_sb[:], mpM_d.ap())
            e3 = cp.tile([K, 3 * NS], f16)
            nc.sync.dma_start(e3[:], e3_d.ap())
            whead = cp.tile([K, 3 * K0_TOT], f16)
            nc.sync.dma_start(whead[:], whead_d.ap())
            bhead = cp.tile([K, 3], f32)
            nc.sync.dma_start(bhead[:], bhead_d.ap())
            wout = cp.tile([K, 3], f16)
            nc.sync.dma_start(wout[:], wout_d.ap())
            bout = cp.tile([1, 1], f32)
            nc.sync.dma_start(bout[:], bout_d.ap())
            rvv = rvin[:].rearrange("p (c t) -> p c t", c=3)

            b_eps = cp.tile([P, 1], f32)
            nc.vector.memset(b_eps[:], 1e-12)
            b_zero = cp.tile([P, 1], f32)
            nc.vector.memset(b_zero[:], 0.0)
            ones14 = cp.tile([P, WIN], f16)
            nc.vector.memset(ones14[:], 1.0)

            # ---- persistent ----
            vt = gp.tile([P, T, 72], f16)
            g_sb = gp.tile([72, NBLK * P], f16)
            outsb = gp.tile([1, NS], f32)

            # ================= pair phase =================
            for ci in range(NCH):
                t0 = ci * TC
                TS = slice(t0, t0 + TC)

                # geometry (rv comes pre-subtracted from the host)
                rv = rvv[:, :, TS]
                rv2 = wp.tile([P, 3, TC], f32, tag="rv2")
                nc.vector.tensor_tensor(out=rv2[:], in0=rv, in1=rv,
                                        op=ALU.mult)
                rr = wp.tile([P, TC], f32, tag="rr")
                nc.vector.tensor_tensor(out=rr[:], in0=rv2[:, 0, :],
                                        in1=rv2[:, 1, :], op=ALU.add)
                nc.vector.tensor_tensor(out=rr[:], in0=rr[:],
                                        in1=rv2[:, 2, :], op=ALU.add)
                lnrr = wp.tile([P, TC], f32, tag="lnrr")
                nc.scalar.activation(lnrr[:], rr[:], AF.Ln,
                                     bias=b_eps[:], scale=1.0)
                invd = wp.tile([P, TC], f32, tag="invd")
                nc.scalar.activation(invd[:], lnrr[:], AF.Exp,
                                     bias=b_zero[:], scale=-0.5)
                dd = wp.tile([P, TC], f32, tag="dd")
                nc.vector.tensor_tensor(out=dd[:], in0=rr[:], in1=invd[:],
                                        op=ALU.mult)
                uv = wp.tile([P, 3, TC], f16, tag="uv")
                nc.vector.tensor_tensor(
                    out=uv[:], in0=rv,
                    in1=invd[:].unsqueeze(1).to_broadcast([P, 3, TC]),
                    op=ALU.mult)
                ux, uy, uz = uv[:, 0, :], uv[:, 1, :], uv[:, 2, :]

                sh = wp.tile([P, 8, TC], f16, tag="sh")
                nc.vector.tensor_copy(sh[:, 0, :], uy)
                nc.vector.tensor_copy(sh[:, 1, :], uz)
                nc.vector.tensor_copy(sh[:, 2, :], ux)
                nc.vector.scalar_tensor_tensor(
                    out=sh[:, 3, :], in0=ux, scalar=SQ3, in1=uy,
                    op0=ALU.mult, op1=ALU.mult)
                nc.vector.scalar_tensor_tensor(
                    out=sh[:, 4, :], in0=uy, scalar=SQ3, in1=uz,
                    op0=ALU.mult, op1=ALU.mult)
                zz3 = wp.tile([P, TC], f16, tag="zz3")
                nc.vector.scalar_tensor_tensor(
                    out=zz3[:], in0=uz, scalar=3.0, in1=uz,
                    op0=ALU.mult, op1=ALU.mult)
                nc.vector.tensor_scalar(
                    out=sh[:, 5, :], in0=zz3[:], scalar1=0.5, scalar2=-0.5,
                    op0=ALU.mult, op1=ALU.add)
                nc.vector.scalar_tensor_tensor(
                    out=sh[:, 6, :], in0=ux, scalar=SQ3, in1=uz,
                    op0=ALU.mult, op1=ALU.mult)
                xx = wp.tile([P, TC], f16, tag="xx")
                nc.vector.scalar_tensor_tensor(
                    out=xx[:], in0=ux, scalar=0.5 * SQ3, in1=ux,
                    op0=ALU.mult, op1=ALU.mult)
                yy = wp.tile([P, TC], f16, tag="yy")
                nc.vector.scalar_tensor_tensor(
                    out=yy[:], in0=uy, scalar=0.5 * SQ3, in1=uy,
                    op0=ALU.mult, op1=ALU.mult)
                nc.vector.tensor_tensor(out=sh[:, 7, :], in0=xx[:],
                                        in1=yy[:], op=ALU.subtract)

                # radial basis [P, TC, 8]
                ev = wp.tile([P, TC, N_BASIS], f16, tag="ev")
                nc.vector.tensor_tensor(
                    out=ev[:],
                    in0=dd[:].unsqueeze(2).to_broadcast([P, TC, N_BASIS]),
                    in1=mu8[:].unsqueeze(1).to_broadcast([P, TC, N_BASIS]),
                    op=ALU.subtract)
                e2 = wp.tile([P, TC, N_BASIS], f16, tag="e2")
                nc.vector.tensor_tensor(out=e2[:], in0=ev[:], in1=ev[:],
                                        op=ALU.mult)
                gauss = wp.tile([P, TC, N_BASIS], f16, tag="gauss")
                nc.scalar.activation(gauss[:], e2[:], AF.Exp,
                                     bias=b_zero[:],
                                     scale=-1.0 / (SIGMA * SIGMA))
                # polynomial cutoff fc
                tq = wp.tile([P, TC], f16, tag="tq")
                nc.vector.tensor_scalar(
                    out=tq[:], in0=dd[:],
                    scalar1=CUTOFF - CUTOFF_WIDTH,
                    scalar2=1.0 / CUTOFF_WIDTH,
                    op0=ALU.subtract, op1=ALU.mult)
                nc.vector.tensor_scalar(
                    out=tq[:], in0=tq[:], scalar1=0.0, scalar2=1.0,
                    op0=ALU.max, op1=ALU.min)
                q = wp.tile([P, TC], f16, tag="q")
                nc.vector.tensor_tensor(out=q[:], in0=tq[:], in1=tq[:],
                                        op=ALU.mult)
                u = wp.tile([P, TC], f16, tag="u")
                nc.vector.tensor_scalar(
                    out=u[:], in0=q[:], scalar1=-FC_C, scalar2=FC_B,
                    op0=ALU.mult, op1=ALU.add)
                w0 = wp.tile([P, TC], f16, tag="w0")
                nc.vector.tensor_tensor(out=w0[:], in0=q[:], in1=u[:],
                                        op=ALU.mult)
                fc = wp.tile([P, TC], f16, tag="fc")
                nc.vector.scalar_tensor_tensor(
                    out=fc[:], in0=w0[:], scalar=-FC_A, in1=q[:],
                    op0=ALU.add, op1=ALU.mult)
                nc.vector.tensor_scalar(
                    out=fc[:], in0=fc[:], scalar1=1.0, scalar2=1.0,
                    op0=ALU.add, op1=ALU.mult)
                rb = wp.tile([P, TC, N_BASIS], f16, tag="rb")
                nc.vector.tensor_tensor(
                    out=rb[:], in0=gauss[:],
                    in1=fc[:].unsqueeze(2).to_broadcast([P, TC, N_BASIS]),
                    op=ALU.mult)

                # vt: [rb | sh x rb]  (outer product all on DVE)
                nc.vector.tensor_copy(vt[:, TS, 0:8], rb[:])
                shr = sh[:].rearrange("p l t -> p t l")
                nc.vector.tensor_tensor(
                    out=vt[:, TS, 8:72].rearrange(
                        "p t (i j) -> p t i j", i=8, j=8),
                    in0=shr[:].unsqueeze(3).to_broadcast([P, TC, 8, 8]),
                    in1=rb[:].unsqueeze(2).to_broadcast([P, TC, 8, 8]),
                    op=ALU.mult)

                # one-hot st [P, TC, 128]
                st = sp.tile([P, TC, P], f16, tag="st")
                if KD > 0:
                    nc.vector.tensor_tensor(
                        out=st[:, 0:KD, :],
                        in0=colf[:, t0:t0 + KD].unsqueeze(2).to_broadcast(
                            [P, KD, P]),
                        in1=iota16[:].unsqueeze(1).to_broadcast([P, KD, P]),
                        op=ALU.is_equal)
                off = KD
                for wi, wt in enumerate(wts):
                    w = ci * len(wts) + wi
                    nc.gpsimd.local_scatter(
                        out_ap=st[:, off:off + wt, :].rearrange(
                            "p t j -> p (t j)"),
                        data_ap=ones14[:, 0:WIN],
                        idxs_ap=idx16[:, w * WIN:(w + 1) * WIN],
                        channels=P,
                        num_elems=wt * P,
                        num_idxs=WIN)
                    off += wt

                # scatter matmuls: 4 blocks per PSUM bank
                for qd in range(CHB // 4):
                    pg = pp.tile([72, 512], f32, space="PSUM", tag="pg")
                    for bl in range(4):
                        b = ci * CHB + qd * 4 + bl
                        for j in range(TPB):
                            trel = (qd * 4 + bl) * TPB + j
                            nc.tensor.matmul(
                                out=pg[:, bl * P:(bl + 1) * P],
                                lhsT=vt[:, t0 + trel, :],
                                rhs=st[:, trel, :],
                                start=(j == 0), stop=(j == TPB - 1))
                    gq = (ci * CHB + qd * 4) * P
                    nc.scalar.copy(g_sb[:, gq:gq + 512], pg[:])

            # ---- G2 shuffle: g_sb [(lm,b),(blk,s,a)] -> g2 [(s,b),(lm,blk,a)]
            g2 = gp.tile([32, 9 * NS], f16)
            g2v4 = g2[:].rearrange("p (lm blk a) -> p lm blk a",
                                   lm=9, blk=NBLK, a=A_BLK)
            qeng = [nc.sync, nc.scalar, nc.gpsimd]
            for lm in range(9):
                gsl8 = g_sb[lm * 8:(lm + 1) * 8, :].rearrange(
                    "b (blk s a) -> b blk s a", s=N_TYPES, a=A_BLK)
                for s in range(N_TYPES):
                    qeng[(lm * 4 + s) % 3].dma_start(
                        g2v4[s * 8:(s + 1) * 8, lm], gsl8[:, :, s, :])

            # ================= atom phase =================
            g2v = g2[:].rearrange("p (lm n) -> p lm n", lm=9)
            POLAR_LMS = (1, 2, 4, 5)
            groups = [(i, min(16, NBLK - i)) for i in range(0, NBLK, 16)]
            for gi, (gb0, gnb) in enumerate(groups):
                n = gnb * A_BLK
                gsl = slice(gb0 * A_BLK, gb0 * A_BLK + n)

                prod = ap.tile([K, 9, 512], f16, tag="prod")
                tl = ap2.tile([K, 3, 512], f16, tag="tl")
                for lm in range(9):
                    polar = lm in POLAR_LMS
                    wA = mpP_sb if polar else mp_sb
                    wB = mpM_sb if polar else mw_sb
                    psA = pp.tile([K, 512], f32, space="PSUM",
                                  tag="ps512", bufs=4)
                    nc.tensor.matmul(out=psA[:, 0:n],
                                     lhsT=wA[:, lm * K:(lm + 1) * K],
                                     rhs=g2v[:, lm, gsl],
                                     start=True, stop=True)
                    psB = pp.tile([K, 512], f32, space="PSUM",
                                  tag="ps512", bufs=4)
                    nc.tensor.matmul(out=psB[:, 0:n],
                                     lhsT=wB[:, lm * K:(lm + 1) * K],
                                     rhs=g2v[:, lm, gsl],
                                     start=True, stop=True)
                    if polar:
                        # prod = psA*psB/4... no: = ((psA/2)^2 - (psB/2)^2)
                        uvt = ap.tile([K, 2, 512], f16, tag="uvt")
                        nc.scalar.activation(uvt[:, 0, 0:n], psA[:, 0:n],
                                             AF.Square, bias=b_zero[:],
                                             scale=0.5)
                        nc.scalar.activation(uvt[:, 1, 0:n], psB[:, 0:n],
                                             AF.Square, bias=b_zero[:],
                                             scale=0.5)
                        nc.vector.tensor_tensor(out=prod[:, lm, 0:n],
                                                in0=uvt[:, 0, 0:n],
                                                in1=uvt[:, 1, 0:n],
                                                op=ALU.subtract)
                    else:
                        # one PSUM operand per op: copy psA (=f) to fp16,
                        # then prod = a16 * psB
                        a16 = ap.tile([K, 512], f16, tag="a16")
                        nc.vector.tensor_copy(a16[:, 0:n], psA[:, 0:n])
                        nc.vector.tensor_tensor(out=prod[:, lm, 0:n],
                                                in0=a16[:, 0:n],
                                                in1=psB[:, 0:n],
                                                op=ALU.mult)
                        if lm == 0:
                            # t0 = f*Wf + f
                            nc.vector.tensor_tensor(out=tl[:, 0, 0:n],
                                                    in0=prod[:, 0, 0:n],
                                                    in1=a16[:, 0:n],
                                                    op=ALU.add)
                # l=1: prod1+prod2+prod3
                ta = ap2.tile([K, 512], f16, tag="ta")
                nc.vector.tensor_tensor(out=ta[:, 0:n], in0=prod[:, 1, 0:n],
                                        in1=prod[:, 2, 0:n], op=ALU.add)
                nc.vector.tensor_tensor(out=tl[:, 1, 0:n], in0=ta[:, 0:n],
                                        in1=prod[:, 3, 0:n], op=ALU.add)
                # l=2: prod4..prod8 (pairwise into prod slots, then combine)
                nc.vector.tensor_tensor(out=prod[:, 4:6, 0:n],
                                        in0=prod[:, 4:6, 0:n],
                                        in1=prod[:, 6:8, 0:n], op=ALU.add)
                nc.vector.tensor_tensor(out=ta[:, 0:n], in0=prod[:, 4, 0:n],
                                        in1=prod[:, 5, 0:n], op=ALU.add)
                nc.vector.tensor_tensor(out=tl[:, 2, 0:n], in0=ta[:, 0:n],
                                        in1=prod[:, 8, 0:n], op=ALU.add)
                # x0e = tl * e3  (one op)
                x0e = ap2.tile([K, 3, 512], f16, tag="x0e")
                e3v = e3[:].rearrange("p (l a) -> p l a", l=3)
                nc.vector.tensor_tensor(out=x0e[:, :, 0:n],
                                        in0=tl[:, :, 0:n],
                                        in1=e3v[:, :, gsl], op=ALU.mult)

                # head
                ht = ap2.tile([K, 3, 512], f16, tag="ht")
                for jc in range(3):
                    psh = pp.tile([K, 512], f32, space="PSUM",
                                  tag="ps512", bufs=4)
                    for rc in range(3):
                        nc.tensor.matmul(
                            out=psh[:, 0:n],
                            lhsT=whead[:, rc * K0_TOT + jc * K:
                                       rc * K0_TOT + (jc + 1) * K],
                            rhs=x0e[:, rc, 0:n],
                            start=(rc == 0), stop=(rc == 2))
                    nc.scalar.activation(ht[:, jc, 0:n], psh[:, 0:n],
                                         AF.Silu,
                                         bias=bhead[:, jc:jc + 1], scale=1.0)
                pso = pp.tile([1, 512], f32, space="PSUM", tag="pso",
                              bufs=2)
                for rc in range(3):
                    nc.tensor.matmul(out=pso[:, 0:n],
                                     lhsT=wout[:, rc:rc + 1],
                                     rhs=ht[:, rc, 0:n],
                                     start=(rc == 0), stop=(rc == 2))
                nc.scalar.activation(outsb[:, gsl], pso[:, 0:n],
                                     AF.Identity, bias=bout[:], scale=1.0)
            nc.sync.dma_start(out_d.ap(), outsb[:])

    nc.compile()
    return nc, T


def _prep_inputs(inputs, TPB):
    T = NBLK * TPB
    TC, KD, wts = _onehot_plan(TPB)
    NW = len(wts) * NCH

    pos = np.ascontiguousarray(np.asarray(inputs["positions"], np.float32))
    spec = np.asarray(inputs["species"]).astype(np.int64)
    pairs = np.asarray(inputs["pairs"]).astype(np.int64)
    ctr, nbr = pairs[:, 0], pairs[:, 1]
    spec_nb = spec[nbr]
    core = ctr // NLOC
    loc = ctr - core * NLOC
    blk = loc // A_BLK
    arel = loc - blk * A_BLK
    order = np.lexsort((arel, spec_nb, blk, core))
    ctr_s, nbr_s = ctr[order], nbr[order]
    spec_s, core_s = spec_nb[order], core[order]
    blk_s, arel_s = blk[order], arel[order]

    key = core_s * NBLK + blk_s
    counts = np.bincount(key, minlength=NCORES * NBLK)
    starts = np.concatenate([[0], np.cumsum(counts)[:-1]])
    rank = np.arange(len(ctr_s)) - starts[key]
    slot = blk_s * (TPB * P) + rank
    tt = slot // P
    qq = slot - tt * P
    colall = spec_s * A_BLK + arel_s          # s-major one-hot column

    iota_np = np.broadcast_to(np.arange(P, dtype=np.float16), (P, P)).copy()
    mu_np = np.broadcast_to(
        np.linspace(0.0, CUTOFF, N_BASIS, dtype=np.float32),
        (P, N_BASIS)).copy()

    emb = np.asarray(inputs["embeddings"], np.float32)
    h0t = np.repeat(emb, N_MAX, axis=1)
    W_rad = np.asarray(inputs["W_rad"], np.float32)
    # packed tables: rows (s*8+b) = 32, cols lm*K+k
    mp = np.zeros((32, 9 * K), np.float32)
    for lm in range(9):
        l = L_OF_LM[lm]
        for s in range(N_TYPES):
            for b in range(N_BASIS):
                mp[s * 8 + b, lm * K:(lm + 1) * K] = \
                    MP_SCALING * W_rad[l, b, :] * h0t[s, :]
    wcg3 = np.stack([
        np.asarray(inputs["W_cg0"], np.float32),
        np.asarray(inputs["W_cg1"], np.float32) * np.float32(-1.0 / SQ3),
        np.asarray(inputs["W_cg2"], np.float32) * np.float32(1.0 / SQ3),
    ], 0)
    I = np.eye(K, dtype=np.float32)
    mw = np.zeros_like(mp)
    mpP = np.zeros_like(mp)
    mpM = np.zeros_like(mp)
    for lm in range(9):
        l = L_OF_LM[lm]
        blk_c = slice(lm * K, (lm + 1) * K)
        mw[:, blk_c] = mp[:, blk_c] @ wcg3[l]
        mpP[:, blk_c] = mp[:, blk_c] @ (wcg3[l] + I)
        mpM[:, blk_c] = mp[:, blk_c] @ (wcg3[l] - I)
    eexp = np.repeat(emb, K0_TOT // N_CHANNELS, axis=1)    # [4, 384]
    W_head = np.asarray(inputs["W_head"], np.float32)
    whead = np.concatenate([W_head[i * K:(i + 1) * K, :] for i in range(3)],
                           axis=1)                          # [128, 3*384]
    b_head = np.asarray(inputs["b_head"], np.float32)
    bhead = b_head.reshape(3, K).T.copy()
    W_out = np.asarray(inputs["W_out"], np.float32)
    wout = W_out[:, 0].reshape(3, K).T.copy()
    bout = np.asarray(inputs["b_out"], np.float32).reshape(1, 1)

    in_maps = []
    for c in range(NCORES):
        m = core_s == c
        rvin = np.zeros((P, 3, T), np.float32)
        rvin[qq[m], :, tt[m]] = pos[nbr_s[m]] - pos[ctr_s[m]]
        colv = np.full((P, T), -1, np.int64)
        colv[qq[m], tt[m]] = colall[m]
        colf = colv.astype(np.float16)        # -1 never matches iota 0..127
        idx16 = np.full((P, max(NW, 1), WIN), -1, np.int16)
        for ci in range(NCH):
            t0 = ci * TC
            off = KD
            for wi, wt in enumerate(wts):
                w = ci * len(wts) + wi
                for j in range(wt):
                    t_abs = t0 + off + j
                    valid = colv[:, t_abs] >= 0
                    idx16[valid, w, j] = (colv[valid, t_abs]
                                          + P * j).astype(np.int16)
                off += wt
        idx16 = idx16.reshape(P, max(NW, 1) * WIN)
        slots = np.arange(NS)
        atom = c * NLOC + np.minimum(slots, NLOC - 1)
        e3 = np.concatenate([eexp[spec[atom]][:, l * K:(l + 1) * K].T
                             for l in range(3)], axis=1)   # [128, 3*NS]
        in_maps.append(dict(
            rvin=rvin.reshape(P, 3 * T), colf=colf, idx16=idx16,
            iota16=iota_np, mu8=mu_np,
            mp=mp.astype(np.float16), mw=mw.astype(np.float16),
            mpP=mpP.astype(np.float16), mpM=mpM.astype(np.float16),
            e3=e3.astype(np.float16), whead=whead.astype(np.float16),
            bhead=bhead, wout=wout.astype(np.float16), bout=bout,
        ))
    return in_maps


def _required_tpb(inputs):
    pairs = np.asarray(inputs["pairs"]).astype(np.int64)
    ctr = pairs[:, 0]
    key = (ctr // NLOC) * NBLK + (ctr % NLOC) // A_BLK
    counts = np.bincount(key, minlength=NCORES * NBLK)
    return max(5, int(math.ceil(counts.max() / P)))


def _install_ntff_hook():
    import types
    if "antenv.axon_hooks" in sys.modules:
        return
    try:
        import antenv
        from trn_agent_boot.trn_boot import _ntff_profile_via_ctypes
        hook = _ntff_profile_via_ctypes("/opt/axon/libaxon_pjrt.so")
        mod = types.ModuleType("antenv.axon_hooks")
        _h = {"hook": hook}
        mod.get_axon_ntff_profile_hook = lambda: _h["hook"]
        mod.set_axon_ntff_profile_hook = lambda h: _h.__setitem__("hook", h)
        sys.modules["antenv.axon_hooks"] = mod
        antenv.axon_hooks = mod
        bass_utils.upload_artifacts = lambda d: f"file://{d}"
    except Exception as e:
        print("ntff hook install failed:", repr(e))


def run_cores(inputs, trace=False):
    if trace:
        _install_ntff_hook()
    TPB = _required_tpb(inputs)
    if TPB not in _BUILD_CACHE:
        _BUILD_CACHE[TPB] = _build(TPB)
    nc, T = _BUILD_CACHE[TPB]
    in_maps = _prep_inputs(inputs, TPB)
    res = bass_utils.run_bass_kernel_spmd(
        nc, in_maps, core_ids=list(range(NCORES)), trace=trace)
    outs = [res.results[c]["out"][0, :NLOC] for c in range(NCORES)]
    full = np.concatenate(outs).reshape(N_ATOMS, 1).astype(np.float32)
    return full, res


def kernel(**inputs):
    full, _ = run_cores(inputs, trace=False)
    return full


# revision 26
# speedup vs baseline: 1.0705x; 1.0705x over previous
"""Trainium2 Bass kernel for nn_BaseModel_2654289789315 (gnn_message_passing).

Restructured v1 (from 160us baseline):
  - Pair phase: geometry in fp16 where 2x DVE modes apply, polynomial cutoff
    (no Sin table), d/invd via one Ln + one Exp, so the whole pair phase uses
    only the {ln,exp} activation-table set. Atom phase uses only {silu,
    identity} -> 2 activation table loads total (baseline had 18).
  - One-hot scatter matrix split between GpSimd (LocalScatter) and DVE
    (is_equal vs iota) so neither engine is the sole bottleneck.
  - s-major one-hot columns (species*32 + atom_rel).
  - Scatter matmuls accumulate 4 blocks per PSUM bank -> 10 batched copies.
  - x0e stage via host-precomputed e3 table (no matmuls, one big DVE mult).
  - Atom phase: PSUM->SBUF copies split scalar/DVE, single big fp16 product,
    tree adds, stt fusions.

Sharding: atoms (and their incident pairs, grouped by center block) sharded
across 8 cores; weights replicated; per-pair endpoint positions materialized
host-side (the "halo exchange" happens at input marshaling).
"""

import sys
if "/opt/trn_rl_repo" not in sys.path:
    sys.path.insert(0, "/opt/trn_rl_repo")

import math
import numpy as np

import concourse.bass as bass
import concourse.mybir as mybir
import concourse.tile as tile
from concourse import bacc, bass_utils

AF = mybir.ActivationFunctionType
ALU = mybir.AluOpType
DT = mybir.dt

# ---- problem constants ----
N_ATOMS = 10000
N_PAIRS = 160000
N_TYPES = 4
N_CHANNELS = 32
N_MAX = 4
N_BASIS = 8
K = 128
CUTOFF = 20.0
CUTOFF_WIDTH = 5.0
MP_SCALING = 0.1
K0_TOT = 384
NCORES = 8
NLOC = N_ATOMS // NCORES          # 1250
A_BLK = 32
NBLK = NLOC // A_BLK + (1 if NLOC % A_BLK else 0)  # 40
NS = NBLK * A_BLK                  # 1280
P = 128
SQ3 = float(np.sqrt(3.0))
SIGMA = CUTOFF / N_BASIS
L_OF_LM = [0, 1, 1, 1, 2, 2, 2, 2, 2]
PI = float(np.pi)
FC_A = PI * PI / 4.0
FC_B = PI ** 4 / 48.0
FC_C = PI ** 6 / 1440.0

NCH = 2                            # pair chunks
CHB = NBLK // NCH                  # 20 blocks per chunk
# one-hot split: first KD tiles of each chunk on DVE, rest on gpsimd
KD_FRAC = 0.17
WIN = 14                           # LocalScatter window (tiles)

_BUILD_CACHE = {}


def _onehot_plan(TPB):
    """Per chunk: KD tiles for DVE is_equal; remaining in gpsimd windows."""
    TC = CHB * TPB
    KD = int(round(TC * KD_FRAC))
    ngp = TC - KD
    nwin = math.ceil(ngp / WIN)
    wts = [min(WIN, ngp - i * WIN) for i in range(nwin)]
    return TC, KD, wts


def _build(TPB):
    T = NBLK * TPB
    TC, KD, wts = _onehot_plan(TPB)
    NW = len(wts) * NCH

    nc = bacc.Bacc("TRN2", target_bir_lowering=False, debug=False,
                   num_devices=NCORES)

    def din(name, shape, dt=DT.float32):
        return nc.dram_tensor(name, shape, dt, kind="ExternalInput")

    rvin_d = din("rvin", [P, 3 * T])
    colf_d = din("colf", [P, T], DT.float16)
    idx16_d = din("idx16", [P, max(NW, 1) * WIN], DT.int16)
    iota16_d = din("iota16", [P, P], DT.float16)
    mu8_d = din("mu8", [P, N_BASIS])
    mp_d = din("mp", [32, 9 * K], DT.float16)
    mw_d = din("mw", [32, 9 * K], DT.float16)
    mpP_d = din("mpP", [32, 9 * K], DT.float16)
    mpM_d = din("mpM", [32, 9 * K], DT.float16)
    e3_d = din("e3", [K, 3 * NS], DT.float16)
    whead_d = din("whead", [K, 3 * K0_TOT], DT.float16)
    bhead_d = din("bhead", [K, 3])
    wout_d = din("wout", [K, 3], DT.float16)
    bout_d = din("bout", [1, 1])
    out_d = nc.dram_tensor("out", [1, NS], DT.float32, kind="ExternalOutput")

    f32 = DT.float32
    f16 = DT.float16

    with tile.TileContext(nc) as tc:
        with tc.tile_pool(name="const", bufs=1) as cp, \
             tc.tile_pool(name="gpool", bufs=1) as gp, \
             tc.tile_pool(name="pair", bufs=2) as wp, \
             tc.tile_pool(name="stp", bufs=2) as sp, \
             tc.tile_pool(name="atom", bufs=2) as ap, \
             tc.tile_pool(name="atom2", bufs=1) as ap2, \
             tc.tile_pool(name="psum", bufs=2, space="PSUM") as pp:

            # ---- inputs first (critical path), then weights ----
            rvin = gp.tile([P, 3 * T], f32)
            nc.sync.dma_start(rvin[:], rvin_d.ap())
            colf = cp.tile([P, T], f16)
            nc.scalar.dma_start(colf[:], colf_d.ap())
            idx16 = cp.tile([P, max(NW, 1) * WIN], DT.int16)
            nc.scalar.dma_start(idx16[:], idx16_d.ap())
            iota16 = cp.tile([P, P], f16)
            nc.scalar.dma_start(iota16[:], iota16_d.ap())
            mu8 = cp.tile([P, N_BASIS], f32)
            nc.sync.dma_start(mu8[:], mu8_d.ap())
            mp_sb = cp.tile([32, 9 * K], f16)
            nc.sync.dma_start(mp_sb[:], mp_d.ap())
            mw_sb = cp.tile([32, 9 * K], f16)
            nc.sync.dma_start(mw_sb[:], mw_d.ap())
            mpP_sb = cp.tile([32, 9 * K], f16)
            nc.sync.dma_start(mpP_sb[:], mpP_d.ap())
            mpM_sb = cp.tile([32, 9 * K], f16)
            nc.sync.dma_start(mpM_sb[:], mpM_d.ap())
            e3 = cp.tile([K, 3 * NS], f16)
            nc.sync.dma_start(e3[:], e3_d.ap())
            whead = cp.tile([K, 3 * K0_TOT], f16)
            nc.sync.dma_start(whead[:], whead_d.ap())
            bhead = cp.tile([K, 3], f32)
            nc.sync.dma_start(bhead[:], bhead_d.ap())
            wout = cp.tile([K, 3], f16)
            nc.sync.dma_start(wout[:], wout_d.ap())
            bout = cp.tile([1, 1], f32)
            nc.sync.dma_start(bout[:], bout_d.ap())
            rvv = rvin[:].rearrange("p (c t) -> p c t", c=3)

            b_eps = cp.tile([P, 1], f32)
            nc.vector.memset(b_eps[:], 1e-12)
            b_zero = cp.tile([P, 1], f32)
            nc.vector.memset(b_zero[:], 0.0)
            ones14 = cp.tile([P, WIN], f16)
            nc.vector.memset(ones14[:], 1.0)

            # ---- persistent ----
            vt = gp.tile([P, T, 72], f16)
            g_sb = gp.tile([72, NBLK * P], f16)   # layout [72, (s, blk, a)]
            gsv2 = g_sb[:].rearrange("p (s blk a) -> p s blk a",
                                     s=N_TYPES, a=A_BLK)
            outsb = gp.tile([1, NS], f32)

            # ================= pair phase =================
            for ci in range(NCH):
                t0 = ci * TC
                TS = slice(t0, t0 + TC)

                # geometry (rv comes pre-subtracted from the host)
                rv = rvv[:, :, TS]
                rv2 = wp.tile([P, 3, TC], f32, tag="rv2")
                nc.vector.tensor_tensor(out=rv2[:], in0=rv, in1=rv,
                                        op=ALU.mult)
                rr = wp.tile([P, TC], f32, tag="rr")
                nc.vector.tensor_tensor(out=rr[:], in0=rv2[:, 0, :],
                                        in1=rv2[:, 1, :], op=ALU.add)
                nc.vector.tensor_tensor(out=rr[:], in0=rr[:],
                                        in1=rv2[:, 2, :], op=ALU.add)
                lnrr = wp.tile([P, TC], f32, tag="lnrr")
                nc.scalar.activation(lnrr[:], rr[:], AF.Ln,
                                     bias=b_eps[:], scale=1.0)
                invd = wp.tile([P, TC], f32, tag="invd")
                nc.scalar.activation(invd[:], lnrr[:], AF.Exp,
                                     bias=b_zero[:], scale=-0.5)
                dd = wp.tile([P, TC], f32, tag="dd")
                nc.vector.tensor_tensor(out=dd[:], in0=rr[:], in1=invd[:],
                                        op=ALU.mult)
                uv = wp.tile([P, 3, TC], f16, tag="uv")
                nc.vector.tensor_tensor(
                    out=uv[:], in0=rv,
                    in1=invd[:].unsqueeze(1).to_broadcast([P, 3, TC]),
                    op=ALU.mult)
                ux, uy, uz = uv[:, 0, :], uv[:, 1, :], uv[:, 2, :]

                sh = wp.tile([P, 8, TC], f16, tag="sh")
                nc.vector.tensor_copy(sh[:, 0, :], uy)
                nc.vector.tensor_copy(sh[:, 1, :], uz)
                nc.vector.tensor_copy(sh[:, 2, :], ux)
                nc.vector.scalar_tensor_tensor(
                    out=sh[:, 3, :], in0=ux, scalar=SQ3, in1=uy,
                    op0=ALU.mult, op1=ALU.mult)
                nc.vector.scalar_tensor_tensor(
                    out=sh[:, 4, :], in0=uy, scalar=SQ3, in1=uz,
                    op0=ALU.mult, op1=ALU.mult)
                zz3 = wp.tile([P, TC], f16, tag="zz3")
                nc.vector.scalar_tensor_tensor(
                    out=zz3[:], in0=uz, scalar=3.0, in1=uz,
                    op0=ALU.mult, op1=ALU.mult)
                nc.vector.tensor_scalar(
                    out=sh[:, 5, :], in0=zz3[:], scalar1=0.5, scalar2=-0.5,
                    op0=ALU.mult, op1=ALU.add)
                nc.vector.scalar_tensor_tensor(
                    out=sh[:, 6, :], in0=ux, scalar=SQ3, in1=uz,
                    op0=ALU.mult, op1=ALU.mult)
                xx = wp.tile([P, TC], f16, tag="xx")
                nc.vector.scalar_tensor_tensor(
                    out=xx[:], in0=ux, scalar=0.5 * SQ3, in1=ux,
                    op0=ALU.mult, op1=ALU.mult)
                yy = wp.tile([P, TC], f16, tag="yy")
                nc.vector.scalar_tensor_tensor(
                    out=yy[:], in0=uy, scalar=0.5 * SQ3, in1=uy,
                    op0=ALU.mult, op1=ALU.mult)
                nc.vector.tensor_tensor(out=sh[:, 7, :], in0=xx[:],
                                        in1=yy[:], op=ALU.subtract)

                # radial basis [P, TC, 8]
                ev = wp.tile([P, TC, N_BASIS], f16, tag="ev")
                nc.vector.tensor_tensor(
                    out=ev[:],
                    in0=dd[:].unsqueeze(2).to_broadcast([P, TC, N_BASIS]),
                    in1=mu8[:].unsqueeze(1).to_broadcast([P, TC, N_BASIS]),
                    op=ALU.subtract)
                e2 = wp.tile([P, TC, N_BASIS], f16, tag="e2")
                nc.vector.tensor_tensor(out=e2[:], in0=ev[:], in1=ev[:],
                                        op=ALU.mult)
                gauss = wp.tile([P, TC, N_BASIS], f16, tag="gauss")
                nc.scalar.activation(gauss[:], e2[:], AF.Exp,
                                     bias=b_zero[:],
                                     scale=-1.0 / (SIGMA * SIGMA))
                # polynomial cutoff fc
                tq = wp.tile([P, TC], f16, tag="tq")
                nc.vector.tensor_scalar(
                    out=tq[:], in0=dd[:],
                    scalar1=CUTOFF - CUTOFF_WIDTH,
                    scalar2=1.0 / CUTOFF_WIDTH,
                    op0=ALU.subtract, op1=ALU.mult)
                nc.vector.tensor_scalar(
                    out=tq[:], in0=tq[:], scalar1=0.0, scalar2=1.0,
                    op0=ALU.max, op1=ALU.min)
                q = wp.tile([P, TC], f16, tag="q")
                nc.vector.tensor_tensor(out=q[:], in0=tq[:], in1=tq[:],
                                        op=ALU.mult)
                u = wp.tile([P, TC], f16, tag="u")
                nc.vector.tensor_scalar(
                    out=u[:], in0=q[:], scalar1=-FC_C, scalar2=FC_B,
                    op0=ALU.mult, op1=ALU.add)
                w0 = wp.tile([P, TC], f16, tag="w0")
                nc.vector.tensor_tensor(out=w0[:], in0=q[:], in1=u[:],
                                        op=ALU.mult)
                fc = wp.tile([P, TC], f16, tag="fc")
                nc.vector.scalar_tensor_tensor(
                    out=fc[:], in0=w0[:], scalar=-FC_A, in1=q[:],
                    op0=ALU.add, op1=ALU.mult)
                nc.vector.tensor_scalar(
                    out=fc[:], in0=fc[:], scalar1=1.0, scalar2=1.0,
                    op0=ALU.add, op1=ALU.mult)
                rb = wp.tile([P, TC, N_BASIS], f16, tag="rb")
                nc.vector.tensor_tensor(
                    out=rb[:], in0=gauss[:],
                    in1=fc[:].unsqueeze(2).to_broadcast([P, TC, N_BASIS]),
                    op=ALU.mult)

                # vt: [rb | sh x rb]  (outer product all on DVE)
                nc.vector.tensor_copy(vt[:, TS, 0:8], rb[:])
                shr = sh[:].rearrange("p l t -> p t l")
                nc.vector.tensor_tensor(
                    out=vt[:, TS, 8:72].rearrange(
                        "p t (i j) -> p t i j", i=8, j=8),
                    in0=shr[:].unsqueeze(3).to_broadcast([P, TC, 8, 8]),
                    in1=rb[:].unsqueeze(2).to_broadcast([P, TC, 8, 8]),
                    op=ALU.mult)

                # one-hot st [P, TC, 128]
                st = sp.tile([P, TC, P], f16, tag="st")
                if KD > 0:
                    nc.vector.tensor_tensor(
                        out=st[:, 0:KD, :],
                        in0=colf[:, t0:t0 + KD].unsqueeze(2).to_broadcast(
                            [P, KD, P]),
                        in1=iota16[:].unsqueeze(1).to_broadcast([P, KD, P]),
                        op=ALU.is_equal)
                off = KD
                for wi, wt in enumerate(wts):
                    w = ci * len(wts) + wi
                    nc.gpsimd.local_scatter(
                        out_ap=st[:, off:off + wt, :].rearrange(
                            "p t j -> p (t j)"),
                        data_ap=ones14[:, 0:WIN],
                        idxs_ap=idx16[:, w * WIN:(w + 1) * WIN],
                        channels=P,
                        num_elems=wt * P,
                        num_idxs=WIN)
                    off += wt

                # scatter matmuls: 4 blocks per PSUM bank.
                # g_sb layout is s-major: [72, (s, blk, a)] so the G2
                # shuffle DMAs move long contiguous runs.
                for qd in range(CHB // 4):
                    pg = pp.tile([72, 512], f32, space="PSUM", tag="pg")
                    for bl in range(4):
                        for j in range(TPB):
                            trel = (qd * 4 + bl) * TPB + j
                            nc.tensor.matmul(
                                out=pg[:, bl * P:(bl + 1) * P],
                                lhsT=vt[:, t0 + trel, :],
                                rhs=st[:, trel, :],
                                start=(j == 0), stop=(j == TPB - 1))
                    qb0 = ci * CHB + qd * 4
                    # pg free = (bl, s, a); dst strided: s*NBLK*32 + blk*32 + a
                    dstv = gsv2[:, :, qb0:qb0 + 4, :].rearrange(
                        "p s blk a -> p blk s a")
                    nc.scalar.copy(dstv, pg[:].rearrange(
                        "p (blk s a) -> p blk s a", s=N_TYPES, a=A_BLK))

            # ---- G2 shuffle: g_sb [(lm,b),(s,blk,a)] -> g2 [(s,b),(lm,blk,a)]
            g2 = gp.tile([32, 9 * NS], f16)
            qeng = [nc.sync, nc.sync, nc.scalar, nc.gpsimd]
            for lm in range(9):
                for s in range(N_TYPES):
                    src = g_sb[lm * 8:(lm + 1) * 8, s * NS:(s + 1) * NS]
                    dst = g2[s * 8:(s + 1) * 8, lm * NS:(lm + 1) * NS]
                    qeng[(lm * 4 + s) % 4].dma_start(dst, src)

            # ================= atom phase =================
            g2v = g2[:].rearrange("p (lm n) -> p lm n", lm=9)
            POLAR_LMS = (1, 2, 4, 5)
            groups = [(i, min(16, NBLK - i)) for i in range(0, NBLK, 16)]
            for gi, (gb0, gnb) in enumerate(groups):
                n = gnb * A_BLK
                gsl = slice(gb0 * A_BLK, gb0 * A_BLK + n)

                prod = ap.tile([K, 9, 512], f16, tag="prod")
                tl = ap2.tile([K, 3, 512], f16, tag="tl")
                for lm in range(9):
                    polar = lm in POLAR_LMS
                    wA = mpP_sb if polar else mp_sb
                    wB = mpM_sb if polar else mw_sb
                    psA = pp.tile([K, 512], f32, space="PSUM",
                                  tag="ps512", bufs=4)
                    nc.tensor.matmul(out=psA[:, 0:n],
                                     lhsT=wA[:, lm * K:(lm + 1) * K],
                                     rhs=g2v[:, lm, gsl],
                                     start=True, stop=True)
                    psB = pp.tile([K, 512], f32, space="PSUM",
                                  tag="ps512", bufs=4)
                    nc.tensor.matmul(out=psB[:, 0:n],
                                     lhsT=wB[:, lm * K:(lm + 1) * K],
                                     rhs=g2v[:, lm, gsl],
                                     start=True, stop=True)
                    if polar:
                        # prod = psA*psB/4... no: = ((psA/2)^2 - (psB/2)^2)
                        uvt = ap.tile([K, 2, 512], f16, tag="uvt")
                        nc.scalar.activation(uvt[:, 0, 0:n], psA[:, 0:n],
                                             AF.Square, bias=b_zero[:],
                                             scale=0.5)
                        nc.scalar.activation(uvt[:, 1, 0:n], psB[:, 0:n],
                                             AF.Square, bias=b_zero[:],
                                             scale=0.5)
                        nc.vector.tensor_tensor(out=prod[:, lm, 0:n],
                                                in0=uvt[:, 0, 0:n],
                                                in1=uvt[:, 1, 0:n],
                                                op=ALU.subtract)
                    else:
                        # one PSUM operand per op: copy psA (=f) to fp16,
                        # then prod = a16 * psB
                        a16 = ap.tile([K, 512], f16, tag="a16")
                        nc.vector.tensor_copy(a16[:, 0:n], psA[:, 0:n])
                        nc.vector.tensor_tensor(out=prod[:, lm, 0:n],
                                                in0=a16[:, 0:n],
                                                in1=psB[:, 0:n],
                                                op=ALU.mult)
                        if lm == 0:
                            # t0 = f*Wf + f
                            nc.vector.tensor_tensor(out=tl[:, 0, 0:n],
                                                    in0=prod[:, 0, 0:n],
                                                    in1=a16[:, 0:n],
                                                    op=ALU.add)
                # l=1: prod1+prod2+prod3
                ta = ap2.tile([K, 512], f16, tag="ta")
                nc.vector.tensor_tensor(out=ta[:, 0:n], in0=prod[:, 1, 0:n],
                                        in1=prod[:, 2, 0:n], op=ALU.add)
                nc.vector.tensor_tensor(out=tl[:, 1, 0:n], in0=ta[:, 0:n],
                                        in1=prod[:, 3, 0:n], op=ALU.add)
                # l=2: prod4..prod8 (pairwise into prod slots, then combine)
                nc.vector.tensor_tensor(out=prod[:, 4:6, 0:n],
                                        in0=prod[:, 4:6, 0:n],
                                        in1=prod[:, 6:8, 0:n], op=ALU.add)
                nc.vector.tensor_tensor(out=ta[:, 0:n], in0=prod[:, 4, 0:n],
                                        in1=prod[:, 5, 0:n], op=ALU.add)
                nc.vector.tensor_tensor(out=tl[:, 2, 0:n], in0=ta[:, 0:n],
                                        in1=prod[:, 8, 0:n], op=ALU.add)
                # x0e = tl * e3  (one op)
                x0e = ap2.tile([K, 3, 512], f16, tag="x0e")
                e3v = e3[:].rearrange("p (l a) -> p l a", l=3)
                nc.vector.tensor_tensor(out=x0e[:, :, 0:n],
                                        in0=tl[:, :, 0:n],
                                        in1=e3v[:, :, gsl], op=ALU.mult)

                # head
                ht = ap2.tile([K, 3, 512], f16, tag="ht")
                for jc in range(3):
                    psh = pp.tile([K, 512], f32, space="PSUM",
                                  tag="ps512", bufs=4)
                    for rc in range(3):
                        nc.tensor.matmul(
                            out=psh[:, 0:n],
                            lhsT=whead[:, rc * K0_TOT + jc * K:
                                       rc * K0_TOT + (jc + 1) * K],
                            rhs=x0e[:, rc, 0:n],
                            start=(rc == 0), stop=(rc == 2))
                    nc.scalar.activation(ht[:, jc, 0:n], psh[:, 0:n],
                                         AF.Silu,
                                         bias=bhead[:, jc:jc + 1], scale=1.0)
                pso = pp.tile([1, 512], f32, space="PSUM", tag="pso",
                              bufs=2)
                for rc in range(3):
                    nc.tensor.matmul(out=pso[:, 0:n],
                                     lhsT=wout[:, rc:rc + 1],
                                     rhs=ht[:, rc, 0:n],
                                     start=(rc == 0), stop=(rc == 2))
                nc.scalar.activation(outsb[:, gsl], pso[:, 0:n],
                                     AF.Identity, bias=bout[:], scale=1.0)
            nc.sync.dma_start(out_d.ap(), outsb[:])

    nc.compile()
    return nc, T


def _prep_inputs(inputs, TPB):
    T = NBLK * TPB
    TC, KD, wts = _onehot_plan(TPB)
    NW = len(wts) * NCH

    pos = np.ascontiguousarray(np.asarray(inputs["positions"], np.float32))
    spec = np.asarray(inputs["species"]).astype(np.int64)
    pairs = np.asarray(inputs["pairs"]).astype(np.int64)
    ctr, nbr = pairs[:, 0], pairs[:, 1]
    spec_nb = spec[nbr]
    core = ctr // NLOC
    loc = ctr - core * NLOC
    blk = loc // A_BLK
    arel = loc - blk * A_BLK
    order = np.lexsort((arel, spec_nb, blk, core))
    ctr_s, nbr_s = ctr[order], nbr[order]
    spec_s, core_s = spec_nb[order], core[order]
    blk_s, arel_s = blk[order], arel[order]

    key = core_s * NBLK + blk_s
    counts = np.bincount(key, minlength=NCORES * NBLK)
    starts = np.concatenate([[0], np.cumsum(counts)[:-1]])
    rank = np.arange(len(ctr_s)) - starts[key]
    slot = blk_s * (TPB * P) + rank
    tt = slot // P
    qq = slot - tt * P
    colall = spec_s * A_BLK + arel_s          # s-major one-hot column

    iota_np = np.broadcast_to(np.arange(P, dtype=np.float16), (P, P)).copy()
    mu_np = np.broadcast_to(
        np.linspace(0.0, CUTOFF, N_BASIS, dtype=np.float32),
        (P, N_BASIS)).copy()

    emb = np.asarray(inputs["embeddings"], np.float32)
    h0t = np.repeat(emb, N_MAX, axis=1)
    W_rad = np.asarray(inputs["W_rad"], np.float32)
    # packed tables: rows (s*8+b) = 32, cols lm*K+k
    mp = np.zeros((32, 9 * K), np.float32)
    for lm in range(9):
        l = L_OF_LM[lm]
        for s in range(N_TYPES):
            for b in range(N_BASIS):
                mp[s * 8 + b, lm * K:(lm + 1) * K] = \
                    MP_SCALING * W_rad[l, b, :] * h0t[s, :]
    wcg3 = np.stack([
        np.asarray(inputs["W_cg0"], np.float32),
        np.asarray(inputs["W_cg1"], np.float32) * np.float32(-1.0 / SQ3),
        np.asarray(inputs["W_cg2"], np.float32) * np.float32(1.0 / SQ3),
    ], 0)
    I = np.eye(K, dtype=np.float32)
    mw = np.zeros_like(mp)
    mpP = np.zeros_like(mp)
    mpM = np.zeros_like(mp)
    for lm in range(9):
        l = L_OF_LM[lm]
        blk_c = slice(lm * K, (lm + 1) * K)
        mw[:, blk_c] = mp[:, blk_c] @ wcg3[l]
        mpP[:, blk_c] = mp[:, blk_c] @ (wcg3[l] + I)
        mpM[:, blk_c] = mp[:, blk_c] @ (wcg3[l] - I)
    eexp = np.repeat(emb, K0_TOT // N_CHANNELS, axis=1)    # [4, 384]
    W_head = np.asarray(inputs["W_head"], np.float32)
    whead = np.concatenate([W_head[i * K:(i + 1) * K, :] for i in range(3)],
                           axis=1)                          # [128, 3*384]
    b_head = np.asarray(inputs["b_head"], np.float32)
    bhead = b_head.reshape(3, K).T.copy()
    W_out = np.asarray(inputs["W_out"], np.float32)
    wout = W_out[:, 0].reshape(3, K).T.copy()
    bout = np.asarray(inputs["b_out"], np.float32).reshape(1, 1)

    in_maps = []
    for c in range(NCORES):
        m = core_s == c
        rvin = np.zeros((P, 3, T), np.float32)
        rvin[qq[m], :, tt[m]] = pos[nbr_s[m]] - pos[ctr_s[m]]
        colv = np.full((P, T), -1, np.int64)
        colv[qq[m], tt[m]] = colall[m]
        colf = colv.astype(np.float16)        # -1 never matches iota 0..127
        idx16 = np.full((P, max(NW, 1), WIN), -1, np.int16)
        for ci in range(NCH):
            t0 = ci * TC
            off = KD
            for wi, wt in enumerate(wts):
                w = ci * len(wts) + wi
                for j in range(wt):
                    t_abs = t0 + off + j
                    valid = colv[:, t_abs] >= 0
                    idx16[valid, w, j] = (colv[valid, t_abs]
                                          + P * j).astype(np.int16)
                off += wt
        idx16 = idx16.reshape(P, max(NW, 1) * WIN)
        slots = np.arange(NS)
        atom = c * NLOC + np.minimum(slots, NLOC - 1)
        e3 = np.concatenate([eexp[spec[atom]][:, l * K:(l + 1) * K].T
                             for l in range(3)], axis=1)   # [128, 3*NS]
        in_maps.append(dict(
            rvin=rvin.reshape(P, 3 * T), colf=colf, idx16=idx16,
            iota16=iota_np, mu8=mu_np,
            mp=mp.astype(np.float16), mw=mw.astype(np.float16),
            mpP=mpP.astype(np.float16), mpM=mpM.astype(np.float16),
            e3=e3.astype(np.float16), whead=whead.astype(np.float16),
            bhead=bhead, wout=wout.astype(np.float16), bout=bout,
        ))
    return in_maps


def _required_tpb(inputs):
    pairs = np.asarray(inputs["pairs"]).astype(np.int64)
    ctr = pairs[:, 0]
    key = (ctr // NLOC) * NBLK + (ctr % NLOC) // A_BLK
    counts = np.bincount(key, minlength=NCORES * NBLK)
    return max(5, int(math.ceil(counts.max() / P)))


def _install_ntff_hook():
    import types
    if "antenv.axon_hooks" in sys.modules:
        return
    try:
        import antenv
        from trn_agent_boot.trn_boot import _ntff_profile_via_ctypes
        hook = _ntff_profile_via_ctypes("/opt/axon/libaxon_pjrt.so")
        mod = types.ModuleType("antenv.axon_hooks")
        _h = {"hook": hook}
        mod.get_axon_ntff_profile_hook = lambda: _h["hook"]
        mod.set_axon_ntff_profile_hook = lambda h: _h.__setitem__("hook", h)
        sys.modules["antenv.axon_hooks"] = mod
        antenv.axon_hooks = mod
        bass_utils.upload_artifacts = lambda d: f"file://{d}"
    except Exception as e:
        print("ntff hook install failed:", repr(e))


def run_cores(inputs, trace=False):
    if trace:
        _install_ntff_hook()
    TPB = _required_tpb(inputs)
    if TPB not in _BUILD_CACHE:
        _BUILD_CACHE[TPB] = _build(TPB)
    nc, T = _BUILD_CACHE[TPB]
    in_maps = _prep_inputs(inputs, TPB)
    res = bass_utils.run_bass_kernel_spmd(
        nc, in_maps, core_ids=list(range(NCORES)), trace=trace)
    outs = [res.results[c]["out"][0, :NLOC] for c in range(NCORES)]
    full = np.concatenate(outs).reshape(N_ATOMS, 1).astype(np.float32)
    return full, res


def kernel(**inputs):
    full, _ = run_cores(inputs, trace=False)
    return full
